# revision 29
# baseline (speedup 1.0000x reference)
"""Trainium2 Bass kernel for nn_CausalTemporalAttention.

Reference computation (B=2, S=2048, H=1024, nh=16, hd=64, block=64):
    qkv = x @ w_qkv ; q,k,v heads
    dots = q k^T * hd^-0.5 ; attn = softmax(dots)            (full row)
    attn = attn * block_causal_mask                          (post-softmax)
    out  = (attn @ v) heads-merged @ w_out + b_out
    attention_weights = attn.mean(heads)
Returns (out [B,S,H], attention_weights [B,S,S]).

Sharding: fully data-parallel over (batch, query-blocks). 8 cores; core c
handles batch c//4 and 8 scattered query blocks of 64 rows chosen so the
block-causal masked area is identical across cores:
    j = c%4 : blocks {j, j+4, j+8, j+12, 19-j, 23-j, 27-j, 31-j}
Sorted ascending, local rank r has global block g <= 4r+3, so the
structural key-prefix 256*(r+1) is core-independent; exact masking is
applied via a tiny per-core {0, 1/16} mask input fused into the softmax
normalize op. One SPMD program, no collectives.

Matmuls use float32r (fp32 data, FP22 reduced-precision read) at full PE
rate; attn is cast to bf16 for the attn@V stage.
"""

import numpy as np
import ml_dtypes
from contextlib import ExitStack

import concourse.bass as bass
import concourse.mybir as mybir
import concourse.tile as tile
from concourse.tile_rust import add_dep_helper
from concourse.bass_utils import run_bass_kernel_spmd
from concourse.masks import make_identity

F32 = mybir.dt.float32
F32R = mybir.dt.float32r
BF16 = mybir.dt.bfloat16

B, S, H = 2, 2048, 1024
NH, HD = 16, 64
NB = 64            # block-causal block size (N_BRAIN_AREAS)
SQ = 512           # query rows per core
NBLK = 8           # local query blocks per core (64 rows each)
N_CORES = 8
SCALE = HD ** -0.5  # 0.125

AluOp = mybir.AluOpType
ActFn = mybir.ActivationFunctionType


def blocks_for_quarter(j: int) -> list[int]:
    """8 global query-block indices for core j (ascending, load-balanced)."""
    return sorted([j, j + 4, j + 8, j + 12, 19 - j, 23 - j, 27 - j, 31 - j])


def _split_excess_waits(nc):
    """Walrus caps sync waits per instruction (1 slot on most opcodes; the
    f32r matmul's fused weight-load in particular). Hoist excess waits onto
    standalone same-engine EventSemaphore instructions (2 slots each)."""
    f = nc.m.functions[0]
    for bb in f.blocks:
        out = []
        for ins in bb.instructions:
            si = ins.sync_info
            nw = len(si.on_wait) if si and si.on_wait else 0
            cap = 2 if ins.opcode == "EventSemaphore" else 1
            if nw > cap:
                waits = list(si.on_wait)
                excess, keep = waits[:-cap], waits[-cap:]
                k = 0
                while excess:
                    chunk, excess = excess[:2], excess[2:]
                    out.append(mybir.InstEventSemaphore(
                        name=f"{ins.name}-wsplit{k}", engine=ins.engine,
                        ins=[], outs=[],
                        sync_info=mybir.SyncInfo(on_wait=chunk, on_update=[])))
                    k += 1
                si.on_wait = keep
            out.append(ins)
        bb.instructions = out


def build_kernel() -> bass.Bass:
    nc = bass.Bass()

    xb = nc.dram_tensor("xb", [S, H], BF16, kind="ExternalInput")
    xq = nc.dram_tensor("xq", [SQ, H], BF16, kind="ExternalInput")
    wqkv = nc.dram_tensor("wqkv", [H, 3 * H], BF16, kind="ExternalInput")
    wout = nc.dram_tensor("wout", [H, H], F32, kind="ExternalInput")  # pre-scaled x16
    bout = nc.dram_tensor("bout", [1, H], F32, kind="ExternalInput")
    ones_in = nc.dram_tensor("ones_in", [1, 128], F32, kind="ExternalInput")
    maskt = nc.dram_tensor("maskt", [S, SQ], BF16, kind="ExternalInput")  # {0,1} transposed mask

    out_q = nc.dram_tensor("out_q", [SQ, H], F32, kind="ExternalOutput")
    aw_q = nc.dram_tensor("aw_q", [SQ, S], F32, kind="ExternalOutput")

    with tile.TileContext(nc) as tc, ExitStack() as ctx:
        # ---------------- persistent pools ----------------
        const = ctx.enter_context(tc.tile_pool(name="const", bufs=1))
        vpool = ctx.enter_context(tc.tile_pool(name="vpool", bufs=1))
        qtpool = ctx.enter_context(tc.tile_pool(name="qtpool", bufs=1))
        otpool = ctx.enter_context(tc.tile_pool(name="otpool", bufs=1))

        ident_b = const.tile([128, 128], BF16)
        make_identity(nc, ident_b)
        ones_col = const.tile([1, 128], F32R)
        nc.sync.dma_start(out=ones_col, in_=ones_in[:, :].bitcast(F32R))
        bout_sb = const.tile([1, H], F32R)
        nc.sync.dma_start(out=bout_sb, in_=bout[:, :].bitcast(F32R))

        # V in natural layout [token, channel], bf16: v_sb[:, t, :] = V[128t:128t+128, :]
        v_sb = vpool.tile([128, 16, H], BF16)
        # Q^T for local queries: qt_sb[:, c, :] = Q^T[128c:128c+128, :SQ]
        qt_sb = qtpool.tile([128, 8, SQ], BF16)
        # K^T resident: kt_sb[:, c, :] = K^T[128c:128c+128, :]  (head h lives in
        # chunk h//2 at partition offset 64*(h%2) -- matches qt_sb's layout)
        kt_sb = qtpool.tile([128, 8, S], BF16)
        # attn@V output, head-transposed: ot_sb[64*(h%2):.., h//2, :] = (attn_h @ V_h)^T
        ot_sb = otpool.tile([128, 8, SQ], F32R)

        # ---------------- phase 1: projections ----------------
        # weight chunks are loaded with ONE large DMA each via a rearranged AP:
        # wqkv[:, c0:c0+n].rearrange("(c p) n -> p c n") -> [128, 8, n] tile
        def _wchunk(pool, tag, c0, n, bufs=None):
            wc = pool.tile([128, 8, n], BF16, tag=tag, name=tag, bufs=bufs)
            nc.sync.dma_start(
                out=wc, in_=wqkv[:, c0:c0 + n].rearrange("(c p) n -> p c n", p=128))
            return wc

        with ExitStack() as p1:
            psum1 = p1.enter_context(tc.tile_pool(name="psum1", bufs=2, space="PSUM"))
            xload = p1.enter_context(tc.tile_pool(name="xload", bufs=5))
            wstr = p1.enter_context(tc.tile_pool(name="wstr", bufs=2))

            def transpose_512(dst, xt_tiles, c, coff):
                """Transpose 4 [128,128] blocks (column c) of 4 row-tiles into
                dst[:, coff:coff+512] via one batched PSUM tile + one copy."""
                ps = psum1.tile([128, 512], BF16, tag="tp")
                for t4 in range(4):
                    nc.tensor.transpose(ps[:, 128 * t4:128 * (t4 + 1)],
                                        xt_tiles[t4][:, 128 * c:128 * (c + 1)], ident_b)
                nc.vector.tensor_copy(dst[:, coff:coff + 512], ps)

            xtfp = p1.enter_context(tc.tile_pool(name="xtfp", bufs=1))

            # --- x^T for the local queries; Q projection ---
            with ExitStack() as pq:
                xtqp = pq.enter_context(tc.tile_pool(name="xtqp", bufs=1))
                xtq = xtqp.tile([128, 8, SQ], BF16)
                xt_tiles = []
                for t4 in range(4):
                    xt = xload.tile([128, H], BF16, tag="xl")
                    nc.sync.dma_start(out=xt, in_=xq[128 * t4:128 * (t4 + 1), :])
                    xt_tiles.append(xt)
                for c in range(8):
                    transpose_512(xtq[:, c, :], xt_tiles, c, 0)
                for qch in range(8):
                    wc = _wchunk(wstr, "wq", 128 * qch, 128)
                    ps = psum1.tile([128, SQ], F32, tag="mm")
                    for c in range(8):
                        nc.tensor.matmul(ps, wc[:, c, :], xtq[:, c, :],
                                         start=(c == 0), stop=(c == 7))
                    nc.scalar.copy(qt_sb[:, qch, :], ps)

            # --- K and V projections, one 512-token chunk at a time ---
            # (x^T chunk rotates; all of wk/wv stays resident)
            wkc = [_wchunk(wstr, f"wk{kch}", H + 128 * kch, 128, bufs=1) for kch in range(8)]
            wvc = [_wchunk(wstr, f"wv{vh}", 2 * H + 512 * vh, 512, bufs=1) for vh in range(2)]
            for tokc in range(4):
                xt_tiles = []
                for t4 in range(4):
                    xt = xload.tile([128, H], BF16, tag="xl")
                    nc.sync.dma_start(out=xt, in_=xb[512 * tokc + 128 * t4:512 * tokc + 128 * (t4 + 1), :])
                    xt_tiles.append(xt)
                xtc = xtfp.tile([128, 8, 512], BF16, name="xtc")
                for c in range(8):
                    transpose_512(xtc[:, c, :], xt_tiles, c, 0)
                # K^T rows for these 512 tokens -> resident SBUF (bf16)
                for kch in range(8):
                    ps = psum1.tile([128, 512], F32, tag="mm")
                    for c in range(8):
                        nc.tensor.matmul(ps, wkc[kch][:, c, :], xtc[:, c, :],
                                         start=(c == 0), stop=(c == 7))
                    nc.vector.tensor_copy(kt_sb[:, kch, 512 * tokc:512 * (tokc + 1)], ps)
                # V rows (natural layout, bf16)
                for t4 in range(4):
                    for vh in range(2):
                        ps = psum1.tile([128, 512], F32, tag="mm")
                        for c in range(8):
                            nc.tensor.matmul(ps, xtc[:, c, 128 * t4:128 * (t4 + 1)], wvc[vh][:, c, :],
                                             start=(c == 0), stop=(c == 7))
                        nc.scalar.copy(v_sb[:, 4 * tokc + t4, 512 * vh:512 * (vh + 1)], ps)

        # ---------------- phase 2: attention ----------------
        with ExitStack() as p2:
            dots_ps = p2.enter_context(tc.tile_pool(name="dots_ps", bufs=2, space="PSUM"))
            mm_ps = p2.enter_context(tc.tile_pool(name="mm_ps", bufs=2, space="PSUM"))
            ot_ps = p2.enter_context(tc.tile_pool(name="ot_ps", bufs=2, space="PSUM"))
            expp = p2.enter_context(tc.tile_pool(name="expp", bufs=3))
            attnp = p2.enter_context(tc.tile_pool(name="attnp", bufs=9))
            atp = p2.enter_context(tc.tile_pool(name="atp", bufs=3))
            sums = p2.enter_context(tc.tile_pool(name="sums", bufs=8))
            awpool = p2.enter_context(tc.tile_pool(name="awpool", bufs=1))
            otstg = p2.enter_context(tc.tile_pool(name="otstg", bufs=2))
            maskp = p2.enter_context(tc.tile_pool(name="maskp", bufs=1))

            # transposed mask: maskt_sb[:, kc, :] = maskt[128kc:128(kc+1), :]
            maskt_sb = maskp.tile([128, 16, SQ], BF16)
            for kc in range(16):
                nc.sync.dma_start(out=maskt_sb[:, kc, :], in_=maskt[128 * kc:128 * (kc + 1), :])

            aw_sb = awpool.tile([128, 4, S], F32)
            ot_dmas = []
            tail_ops = []

            def head_front(h):
                """QK -> exp -> normalize -> aw accumulation for head h."""
                hc, hp = h // 2, h % 2
                kht = kt_sb[64 * hp:64 * (hp + 1), hc, :]

                attn_q = []
                for qc in range(4):
                    pq = 512 * (qc + 1)  # structural prefix for this q-chunk
                    qt_h = qt_sb[64 * hp:64 * (hp + 1), hc, 128 * qc:128 * (qc + 1)]
                    halves = []
                    for half in range(2):
                        d = dots_ps.tile([128, 1024], F32, tag="dots")
                        for k2 in range(2):
                            nc.tensor.matmul(
                                d[:, 512 * k2:512 * (k2 + 1)], (qt_h),
                                (kht[:, 1024 * half + 512 * k2:1024 * half + 512 * (k2 + 1)]),
                                start=True, stop=True)
                        ex = expp.tile([128, 1024], BF16, tag="exp")
                        sacc = sums.tile([128, 1], F32, tag="s")
                        nc.scalar.activation(ex, d, ActFn.Exp, scale=SCALE, accum_out=sacc)
                        halves.append((ex, sacc))
                    stot = sums.tile([128, 1], F32, tag="s")
                    nc.vector.tensor_add(stot, halves[0][1], halves[1][1])
                    recip = sums.tile([128, 1], F32, tag="s")
                    nc.vector.reciprocal(recip, stot)

                    # attn = exp * recip / 16 over the (uniform) structural
                    # prefix; true block-causal masking is fused into the
                    # attn^T evacuation in head_back. attention-weight rows
                    # beyond the true prefix are discarded host-side.
                    attn = attnp.tile([128, S], BF16)
                    for half in range(2):
                        c0, c1 = 1024 * half, min(pq, 1024 * (half + 1))
                        if c1 <= c0:
                            continue
                        nc.vector.tensor_scalar(
                            out=attn[:, c0:c1], in0=halves[half][0][:, 0:c1 - c0],
                            scalar1=recip, scalar2=1.0 / 16.0,
                            op0=AluOp.mult, op1=AluOp.mult)
                    # attention-weight accumulation (mean over heads)
                    eng = nc.gpsimd if qc < 2 else nc.vector
                    awdst = aw_sb[:, qc, 0:pq]
                    if h == 0:
                        op = eng.tensor_copy(awdst, attn[:, 0:pq])
                    else:
                        op = eng.tensor_add(awdst, attn[:, 0:pq], awdst)
                    if h == NH - 1:
                        tail_ops.append(op)
                    attn_q.append(attn)
                return attn_q

            def head_back(h, attn_q):
                """attn^T (with fused mask) and attn @ V for head h."""
                hc, hp = h // 2, h % 2
                otps = ot_ps.tile([64, SQ], F32, tag="ot")
                for kc in range(16):
                    qc_min = max(0, (kc - 3 + 3) // 4)  # ceil((kc-3)/4)
                    n0 = 128 * qc_min
                    at_ps = mm_ps.tile([128, SQ], BF16, tag="at")
                    for qc in range(qc_min, 4):
                        nc.tensor.transpose(at_ps[:, 128 * qc:128 * (qc + 1)],
                                            attn_q[qc][:, 128 * kc:128 * (kc + 1)], ident_b)
                    at = atp.tile([128, SQ], BF16)
                    nc.vector.tensor_tensor(at[:, n0:SQ], at_ps[:, n0:SQ],
                                            maskt_sb[:, kc, n0:SQ], op=AluOp.mult)
                    nc.tensor.matmul(otps[:, n0:SQ], v_sb[:, kc, 64 * h:64 * (h + 1)],
                                     at[:, n0:SQ], start=(kc == 0), stop=(kc == 15),
                                     skip_group_check=True)
                # evacuate O^T via DMA (engine copies cannot shift partitions)
                otst = otstg.tile([64, SQ], F32R)
                nc.scalar.copy(otst, otps)
                ot_dmas.append(nc.sync.dma_start(out=ot_sb[64 * hp:64 * (hp + 1), hc, :], in_=otst))

            prev = None
            for h in range(NH):
                attn_q = head_front(h)
                if prev is not None:
                    head_back(prev[0], prev[1])
                prev = (h, attn_q)
            head_back(prev[0], prev[1])

            # attention weights out
            for qc in range(4):
                tail_ops.append(nc.sync.dma_start(out=aw_q[128 * qc:128 * (qc + 1), :], in_=aw_sb[:, qc, :]))

        # ---------------- phase 3: output projection ----------------
        with ExitStack() as p3:
            psum3 = p3.enter_context(tc.tile_pool(name="psum3", bufs=4, space="PSUM"))
            wstr3 = p3.enter_context(tc.tile_pool(name="wstr3", bufs=2))
            stg3 = p3.enter_context(tc.tile_pool(name="stg3", bufs=3))
            for n2 in range(2):
                pss = []
                for tcn in range(4):
                    ps = psum3.tile([128, 512], F32, tag="op")
                    nc.tensor.matmul(ps, (ones_col), (bout_sb[:, 512 * n2:512 * (n2 + 1)]),
                                     start=True, stop=False, skip_group_check=True)
                    pss.append(ps)
                w = wstr3.tile([128, 8, 512], F32R)
                nc.sync.dma_start(
                    out=w, in_=wout[:, 512 * n2:512 * (n2 + 1)].rearrange("(c p) n -> p c n", p=128).bitcast(F32R))
                for c in range(8):
                    for tcn in range(4):
                        nc.tensor.matmul(pss[tcn], (ot_sb[:, c, 128 * tcn:128 * (tcn + 1)]),
                                         (w[:, c, :]), start=False, stop=(c == 7), skip_group_check=True)
                for tcn in range(4):
                    st = stg3.tile([128, 512], F32)
                    nc.scalar.copy(st, pss[tcn])
                    nc.sync.dma_start(out=out_q[128 * tcn:128 * (tcn + 1), 512 * n2:512 * (n2 + 1)], in_=st)

    _split_excess_waits(nc)
    return nc


_COMPILED = None


def _get_nc():
    global _COMPILED
    if _COMPILED is None:
        _COMPILED = build_kernel()
    return _COMPILED


def _host_maskt(j: int) -> np.ndarray:
    """[S, SQ] bf16 transposed mask: 1 where key-block <= global query-block."""
    g = np.repeat(np.array(blocks_for_quarter(j)), NB)        # [SQ] global block per local col
    kb = np.arange(S) // NB                                   # [S] key block per row
    return (kb[:, None] <= g[None, :]).astype(ml_dtypes.bfloat16)


def run_shards(x, w_qkv, w_out, b_out, **spmd_kwargs):
    """Shard inputs, run the SPMD kernel, reassemble. Returns (output, attnw, bkr)."""
    x = np.ascontiguousarray(np.asarray(x, dtype=np.float32).astype(ml_dtypes.bfloat16))
    w_qkv = np.ascontiguousarray(np.asarray(w_qkv, dtype=np.float32).astype(ml_dtypes.bfloat16))
    w_out16 = np.ascontiguousarray(np.asarray(w_out, dtype=np.float32) * 16.0)
    b_out = np.ascontiguousarray(np.asarray(b_out, dtype=np.float32)).reshape(1, H)

    nc = _get_nc()
    in_maps = []
    for c in range(N_CORES):
        b, j = c // 4, c % 4
        rows = np.concatenate([np.arange(NB * g, NB * (g + 1)) for g in blocks_for_quarter(j)])
        in_maps.append({
            "xb": x[b],
            "xq": np.ascontiguousarray(x[b][rows]),
            "wqkv": w_qkv,
            "wout": w_out16,
            "bout": b_out,
            "ones_in": np.ones((1, 128), np.float32),
            "maskt": _host_maskt(j),
        })
    res = run_bass_kernel_spmd(nc, in_maps, core_ids=list(range(N_CORES)), **spmd_kwargs)

    output = np.empty((B, S, H), dtype=np.float32)
    attnw = np.zeros((B, S, S), dtype=np.float32)
    for c in range(N_CORES):
        b, j = c // 4, c % 4
        o = np.asarray(res.results[c]["out_q"])
        a = np.asarray(res.results[c]["aw_q"])
        for r, g in enumerate(blocks_for_quarter(j)):
            rows = slice(NB * g, NB * (g + 1))
            output[b, rows, :] = o[NB * r:NB * (r + 1), :]
            pref = NB * (g + 1)
            attnw[b, rows, :pref] = a[NB * r:NB * (r + 1), :pref]
    return output, attnw, res


def kernel(x, w_qkv, w_out, b_out):
    output, attnw, _ = run_shards(x, w_qkv, w_out, b_out)
    return output, attnw


# revision 31
# speedup vs baseline: 890.0097x; 890.0097x over previous
"""Trainium2 Bass kernel for nn_CausalTemporalAttention.

Reference computation (B=2, S=2048, H=1024, nh=16, hd=64, block=64):
    qkv = x @ w_qkv ; q,k,v heads
    dots = q k^T * hd^-0.5 ; attn = softmax(dots)            (full row)
    attn = attn * block_causal_mask                          (post-softmax)
    out  = (attn @ v) heads-merged @ w_out + b_out
    attention_weights = attn.mean(heads)
Returns (out [B,S,H], attention_weights [B,S,S]).

Sharding: fully data-parallel over (batch, query-blocks). 8 cores; core c
handles batch c//4 and 8 scattered query blocks of 64 rows chosen so the
block-causal masked area is identical across cores:
    j = c%4 : blocks {j, j+4, j+8, j+12, 19-j, 23-j, 27-j, 31-j}
Sorted ascending, local rank r has global block g <= 4r+3, so the
structural key-prefix 256*(r+1) is core-independent; exact masking is
applied via a tiny per-core {0, 1/16} mask input fused into the softmax
normalize op. One SPMD program, no collectives.

Matmuls use float32r (fp32 data, FP22 reduced-precision read) at full PE
rate; attn is cast to bf16 for the attn@V stage.
"""

import numpy as np
import ml_dtypes
from contextlib import ExitStack

import concourse.bass as bass
import concourse.mybir as mybir
import concourse.tile as tile
from concourse.tile_rust import add_dep_helper
from concourse.bass_utils import run_bass_kernel_spmd
from concourse.masks import make_identity

F32 = mybir.dt.float32
F32R = mybir.dt.float32r
BF16 = mybir.dt.bfloat16

B, S, H = 2, 2048, 1024
NH, HD = 16, 64
NB = 64            # block-causal block size (N_BRAIN_AREAS)
SQ = 512           # query rows per core
NBLK = 8           # local query blocks per core (64 rows each)
N_CORES = 8
SCALE = HD ** -0.5  # 0.125

AluOp = mybir.AluOpType
ActFn = mybir.ActivationFunctionType


def blocks_for_quarter(j: int) -> list[int]:
    """8 global query-block indices for core j (ascending, load-balanced)."""
    return sorted([j, j + 4, j + 8, j + 12, 19 - j, 23 - j, 27 - j, 31 - j])


def _split_excess_waits(nc):
    """Walrus caps sync waits per instruction (1 slot on most opcodes; the
    f32r matmul's fused weight-load in particular). Hoist excess waits onto
    standalone same-engine EventSemaphore instructions (2 slots each)."""
    f = nc.m.functions[0]
    for bb in f.blocks:
        out = []
        for ins in bb.instructions:
            si = ins.sync_info
            nw = len(si.on_wait) if si and si.on_wait else 0
            cap = 2 if ins.opcode == "EventSemaphore" else 1
            if nw > cap:
                waits = list(si.on_wait)
                excess, keep = waits[:-cap], waits[-cap:]
                k = 0
                while excess:
                    chunk, excess = excess[:2], excess[2:]
                    out.append(mybir.InstEventSemaphore(
                        name=f"{ins.name}-wsplit{k}", engine=ins.engine,
                        ins=[], outs=[],
                        sync_info=mybir.SyncInfo(on_wait=chunk, on_update=[])))
                    k += 1
                si.on_wait = keep
            out.append(ins)
        bb.instructions = out


def build_kernel(nrep: int = 1) -> bass.Bass:
    nc = bass.Bass()

    xb = nc.dram_tensor("xb", [S, H], BF16, kind="ExternalInput")
    xq = nc.dram_tensor("xq", [SQ, H], BF16, kind="ExternalInput")
    wqkv = nc.dram_tensor("wqkv", [H, 3 * H], BF16, kind="ExternalInput")
    wout = nc.dram_tensor("wout", [H, H], F32, kind="ExternalInput")  # pre-scaled x16
    bout = nc.dram_tensor("bout", [1, H], F32, kind="ExternalInput")
    ones_in = nc.dram_tensor("ones_in", [1, 128], F32, kind="ExternalInput")
    maskt = nc.dram_tensor("maskt", [S, SQ], BF16, kind="ExternalInput")  # {0,1} transposed mask

    out_q = nc.dram_tensor("out_q", [SQ, H], F32, kind="ExternalOutput")
    aw_q = nc.dram_tensor("aw_q", [SQ, S], F32, kind="ExternalOutput")

    with tile.TileContext(nc) as tc:
     for _rep in range(nrep):
      with ExitStack() as ctx:
        # ---------------- persistent pools ----------------
        const = ctx.enter_context(tc.tile_pool(name="const", bufs=1))
        vpool = ctx.enter_context(tc.tile_pool(name="vpool", bufs=1))
        qtpool = ctx.enter_context(tc.tile_pool(name="qtpool", bufs=1))
        otpool = ctx.enter_context(tc.tile_pool(name="otpool", bufs=1))

        ident_b = const.tile([128, 128], BF16)
        make_identity(nc, ident_b)
        ones_col = const.tile([1, 128], F32R)
        nc.sync.dma_start(out=ones_col, in_=ones_in[:, :].bitcast(F32R))
        bout_sb = const.tile([1, H], F32R)
        nc.sync.dma_start(out=bout_sb, in_=bout[:, :].bitcast(F32R))

        # V in natural layout [token, channel], bf16: v_sb[:, t, :] = V[128t:128t+128, :]
        v_sb = vpool.tile([128, 16, H], BF16)
        # Q^T for local queries: qt_sb[:, c, :] = Q^T[128c:128c+128, :SQ]
        qt_sb = qtpool.tile([128, 8, SQ], BF16)
        # K^T resident: kt_sb[:, c, :] = K^T[128c:128c+128, :]  (head h lives in
        # chunk h//2 at partition offset 64*(h%2) -- matches qt_sb's layout)
        kt_sb = qtpool.tile([128, 8, S], BF16)
        # attn@V output, head-transposed: ot_sb[64*(h%2):.., h//2, :] = (attn_h @ V_h)^T
        ot_sb = otpool.tile([128, 8, SQ], F32R)

        # ---------------- phase 1: projections ----------------
        # weight chunks are loaded with ONE large DMA each via a rearranged AP:
        # wqkv[:, c0:c0+n].rearrange("(c p) n -> p c n") -> [128, 8, n] tile
        def _wchunk(pool, tag, c0, n, bufs=None):
            wc = pool.tile([128, 8, n], BF16, tag=tag, name=tag, bufs=bufs)
            nc.sync.dma_start(
                out=wc, in_=wqkv[:, c0:c0 + n].rearrange("(c p) n -> p c n", p=128))
            return wc

        with ExitStack() as p1:
            psum1 = p1.enter_context(tc.tile_pool(name="psum1", bufs=2, space="PSUM"))
            xload = p1.enter_context(tc.tile_pool(name="xload", bufs=5))
            wstr = p1.enter_context(tc.tile_pool(name="wstr", bufs=2))

            def transpose_512(dst, xt_tiles, c, coff):
                """Transpose 4 [128,128] blocks (column c) of 4 row-tiles into
                dst[:, coff:coff+512] via one batched PSUM tile + one copy."""
                ps = psum1.tile([128, 512], BF16, tag="tp")
                for t4 in range(4):
                    nc.tensor.transpose(ps[:, 128 * t4:128 * (t4 + 1)],
                                        xt_tiles[t4][:, 128 * c:128 * (c + 1)], ident_b)
                nc.vector.tensor_copy(dst[:, coff:coff + 512], ps)

            xtfp = p1.enter_context(tc.tile_pool(name="xtfp", bufs=1))

            # --- x^T for the local queries; Q projection ---
            with ExitStack() as pq:
                xtqp = pq.enter_context(tc.tile_pool(name="xtqp", bufs=1))
                xtq = xtqp.tile([128, 8, SQ], BF16)
                xt_tiles = []
                for t4 in range(4):
                    xt = xload.tile([128, H], BF16, tag="xl")
                    nc.sync.dma_start(out=xt, in_=xq[128 * t4:128 * (t4 + 1), :])
                    xt_tiles.append(xt)
                for c in range(8):
                    transpose_512(xtq[:, c, :], xt_tiles, c, 0)
                for qch in range(8):
                    wc = _wchunk(wstr, "wq", 128 * qch, 128)
                    ps = psum1.tile([128, SQ], F32, tag="mm")
                    for c in range(8):
                        nc.tensor.matmul(ps, wc[:, c, :], xtq[:, c, :],
                                         start=(c == 0), stop=(c == 7))
                    nc.scalar.copy(qt_sb[:, qch, :], ps)

            # --- K and V projections, one 512-token chunk at a time ---
            # (x^T chunk rotates; all of wk/wv stays resident)
            wkc = [_wchunk(wstr, f"wk{kch}", H + 128 * kch, 128, bufs=1) for kch in range(8)]
            wvc = [_wchunk(wstr, f"wv{vh}", 2 * H + 512 * vh, 512, bufs=1) for vh in range(2)]
            for tokc in range(4):
                xt_tiles = []
                for t4 in range(4):
                    xt = xload.tile([128, H], BF16, tag="xl")
                    nc.sync.dma_start(out=xt, in_=xb[512 * tokc + 128 * t4:512 * tokc + 128 * (t4 + 1), :])
                    xt_tiles.append(xt)
                xtc = xtfp.tile([128, 8, 512], BF16, name="xtc")
                for c in range(8):
                    transpose_512(xtc[:, c, :], xt_tiles, c, 0)
                # K^T rows for these 512 tokens -> resident SBUF (bf16)
                for kch in range(8):
                    ps = psum1.tile([128, 512], F32, tag="mm")
                    for c in range(8):
                        nc.tensor.matmul(ps, wkc[kch][:, c, :], xtc[:, c, :],
                                         start=(c == 0), stop=(c == 7))
                    nc.vector.tensor_copy(kt_sb[:, kch, 512 * tokc:512 * (tokc + 1)], ps)
                # V rows (natural layout, bf16)
                for t4 in range(4):
                    for vh in range(2):
                        ps = psum1.tile([128, 512], F32, tag="mm")
                        for c in range(8):
                            nc.tensor.matmul(ps, xtc[:, c, 128 * t4:128 * (t4 + 1)], wvc[vh][:, c, :],
                                             start=(c == 0), stop=(c == 7))
                        nc.scalar.copy(v_sb[:, 4 * tokc + t4, 512 * vh:512 * (vh + 1)], ps)

        # ---------------- phase 2: attention ----------------
        with ExitStack() as p2:
            dots_ps = p2.enter_context(tc.tile_pool(name="dots_ps", bufs=2, space="PSUM"))
            mm_ps = p2.enter_context(tc.tile_pool(name="mm_ps", bufs=2, space="PSUM"))
            ot_ps = p2.enter_context(tc.tile_pool(name="ot_ps", bufs=2, space="PSUM"))
            expp = p2.enter_context(tc.tile_pool(name="expp", bufs=3))
            attnp = p2.enter_context(tc.tile_pool(name="attnp", bufs=9))
            atp = p2.enter_context(tc.tile_pool(name="atp", bufs=3))
            sums = p2.enter_context(tc.tile_pool(name="sums", bufs=8))
            awpool = p2.enter_context(tc.tile_pool(name="awpool", bufs=1))
            otstg = p2.enter_context(tc.tile_pool(name="otstg", bufs=2))
            maskp = p2.enter_context(tc.tile_pool(name="maskp", bufs=1))

            # transposed mask: maskt_sb[:, kc, :] = maskt[128kc:128(kc+1), :]
            maskt_sb = maskp.tile([128, 16, SQ], BF16)
            for kc in range(16):
                nc.sync.dma_start(out=maskt_sb[:, kc, :], in_=maskt[128 * kc:128 * (kc + 1), :])

            aw_sb = awpool.tile([128, 4, S], F32)
            ot_dmas = []
            tail_ops = []

            def head_front(h):
                """QK -> exp -> normalize -> aw accumulation for head h."""
                hc, hp = h // 2, h % 2
                kht = kt_sb[64 * hp:64 * (hp + 1), hc, :]

                attn_q = []
                for qc in range(4):
                    pq = 512 * (qc + 1)  # structural prefix for this q-chunk
                    qt_h = qt_sb[64 * hp:64 * (hp + 1), hc, 128 * qc:128 * (qc + 1)]
                    halves = []
                    for half in range(2):
                        d = dots_ps.tile([128, 1024], F32, tag="dots")
                        for k2 in range(2):
                            nc.tensor.matmul(
                                d[:, 512 * k2:512 * (k2 + 1)], (qt_h),
                                (kht[:, 1024 * half + 512 * k2:1024 * half + 512 * (k2 + 1)]),
                                start=True, stop=True)
                        ex = expp.tile([128, 1024], BF16, tag="exp")
                        sacc = sums.tile([128, 1], F32, tag="s")
                        nc.scalar.activation(ex, d, ActFn.Exp, scale=SCALE, accum_out=sacc)
                        halves.append((ex, sacc))
                    stot = sums.tile([128, 1], F32, tag="s")
                    nc.vector.tensor_add(stot, halves[0][1], halves[1][1])
                    recip = sums.tile([128, 1], F32, tag="s")
                    nc.vector.reciprocal(recip, stot)

                    # attn = exp * recip / 16 over the (uniform) structural
                    # prefix; true block-causal masking is fused into the
                    # attn^T evacuation in head_back. attention-weight rows
                    # beyond the true prefix are discarded host-side.
                    attn = attnp.tile([128, S], BF16)
                    for half in range(2):
                        c0, c1 = 1024 * half, min(pq, 1024 * (half + 1))
                        if c1 <= c0:
                            continue
                        nc.vector.tensor_scalar(
                            out=attn[:, c0:c1], in0=halves[half][0][:, 0:c1 - c0],
                            scalar1=recip, scalar2=1.0 / 16.0,
                            op0=AluOp.mult, op1=AluOp.mult)
                    # attention-weight accumulation (mean over heads)
                    eng = nc.gpsimd if qc < 2 else nc.vector
                    awdst = aw_sb[:, qc, 0:pq]
                    if h == 0:
                        op = eng.tensor_copy(awdst, attn[:, 0:pq])
                    else:
                        op = eng.tensor_add(awdst, attn[:, 0:pq], awdst)
                    if h == NH - 1:
                        tail_ops.append(op)
                    attn_q.append(attn)
                return attn_q

            def head_back(h, attn_q):
                """attn^T (with fused mask) and attn @ V for head h."""
                hc, hp = h // 2, h % 2
                otps = ot_ps.tile([64, SQ], F32, tag="ot")
                for kc in range(16):
                    qc_min = max(0, (kc - 3 + 3) // 4)  # ceil((kc-3)/4)
                    n0 = 128 * qc_min
                    at_ps = mm_ps.tile([128, SQ], BF16, tag="at")
                    for qc in range(qc_min, 4):
                        nc.tensor.transpose(at_ps[:, 128 * qc:128 * (qc + 1)],
                                            attn_q[qc][:, 128 * kc:128 * (kc + 1)], ident_b)
                    at = atp.tile([128, SQ], BF16)
                    nc.vector.tensor_tensor(at[:, n0:SQ], at_ps[:, n0:SQ],
                                            maskt_sb[:, kc, n0:SQ], op=AluOp.mult)
                    nc.tensor.matmul(otps[:, n0:SQ], v_sb[:, kc, 64 * h:64 * (h + 1)],
                                     at[:, n0:SQ], start=(kc == 0), stop=(kc == 15),
                                     skip_group_check=True)
                # evacuate O^T via DMA (engine copies cannot shift partitions)
                otst = otstg.tile([64, SQ], F32R)
                nc.scalar.copy(otst, otps)
                ot_dmas.append(nc.sync.dma_start(out=ot_sb[64 * hp:64 * (hp + 1), hc, :], in_=otst))

            prev = None
            for h in range(NH):
                attn_q = head_front(h)
                if prev is not None:
                    head_back(prev[0], prev[1])
                prev = (h, attn_q)
            head_back(prev[0], prev[1])

            # attention weights out
            for qc in range(4):
                tail_ops.append(nc.sync.dma_start(out=aw_q[128 * qc:128 * (qc + 1), :], in_=aw_sb[:, qc, :]))

        # ---------------- phase 3: output projection ----------------
        with ExitStack() as p3:
            psum3 = p3.enter_context(tc.tile_pool(name="psum3", bufs=4, space="PSUM"))
            wstr3 = p3.enter_context(tc.tile_pool(name="wstr3", bufs=2))
            stg3 = p3.enter_context(tc.tile_pool(name="stg3", bufs=3))
            for n2 in range(2):
                pss = []
                for tcn in range(4):
                    ps = psum3.tile([128, 512], F32, tag="op")
                    nc.tensor.matmul(ps, (ones_col), (bout_sb[:, 512 * n2:512 * (n2 + 1)]),
                                     start=True, stop=False, skip_group_check=True)
                    pss.append(ps)
                w = wstr3.tile([128, 8, 512], F32R)
                nc.sync.dma_start(
                    out=w, in_=wout[:, 512 * n2:512 * (n2 + 1)].rearrange("(c p) n -> p c n", p=128).bitcast(F32R))
                for c in range(8):
                    for tcn in range(4):
                        nc.tensor.matmul(pss[tcn], (ot_sb[:, c, 128 * tcn:128 * (tcn + 1)]),
                                         (w[:, c, :]), start=False, stop=(c == 7), skip_group_check=True)
                for tcn in range(4):
                    st = stg3.tile([128, 512], F32)
                    nc.scalar.copy(st, pss[tcn])
                    nc.sync.dma_start(out=out_q[128 * tcn:128 * (tcn + 1), 512 * n2:512 * (n2 + 1)], in_=st)

    _split_excess_waits(nc)
    return nc


_COMPILED = None


def _get_nc():
    global _COMPILED
    if _COMPILED is None:
        _COMPILED = build_kernel()
    return _COMPILED


def _host_maskt(j: int) -> np.ndarray:
    """[S, SQ] bf16 transposed mask: 1 where key-block <= global query-block."""
    g = np.repeat(np.array(blocks_for_quarter(j)), NB)        # [SQ] global block per local col
    kb = np.arange(S) // NB                                   # [S] key block per row
    return (kb[:, None] <= g[None, :]).astype(ml_dtypes.bfloat16)


def run_shards(x, w_qkv, w_out, b_out, **spmd_kwargs):
    """Shard inputs, run the SPMD kernel, reassemble. Returns (output, attnw, bkr)."""
    x = np.ascontiguousarray(np.asarray(x, dtype=np.float32).astype(ml_dtypes.bfloat16))
    w_qkv = np.ascontiguousarray(np.asarray(w_qkv, dtype=np.float32).astype(ml_dtypes.bfloat16))
    w_out16 = np.ascontiguousarray(np.asarray(w_out, dtype=np.float32) * 16.0)
    b_out = np.ascontiguousarray(np.asarray(b_out, dtype=np.float32)).reshape(1, H)

    nc = _get_nc()
    in_maps = []
    for c in range(N_CORES):
        b, j = c // 4, c % 4
        rows = np.concatenate([np.arange(NB * g, NB * (g + 1)) for g in blocks_for_quarter(j)])
        in_maps.append({
            "xb": x[b],
            "xq": np.ascontiguousarray(x[b][rows]),
            "wqkv": w_qkv,
            "wout": w_out16,
            "bout": b_out,
            "ones_in": np.ones((1, 128), np.float32),
            "maskt": _host_maskt(j),
        })
    res = run_bass_kernel_spmd(nc, in_maps, core_ids=list(range(N_CORES)), **spmd_kwargs)

    output = np.empty((B, S, H), dtype=np.float32)
    attnw = np.zeros((B, S, S), dtype=np.float32)
    for c in range(N_CORES):
        b, j = c // 4, c % 4
        o = np.asarray(res.results[c]["out_q"])
        a = np.asarray(res.results[c]["aw_q"])
        for r, g in enumerate(blocks_for_quarter(j)):
            rows = slice(NB * g, NB * (g + 1))
            output[b, rows, :] = o[NB * r:NB * (r + 1), :]
            pref = NB * (g + 1)
            attnw[b, rows, :pref] = a[NB * r:NB * (r + 1), :pref]
    return output, attnw, res


def kernel(x, w_qkv, w_out, b_out):
    output, attnw, _ = run_shards(x, w_qkv, w_out, b_out)
    return output, attnw


# revision 33
# speedup vs baseline: 27723.0514x; 31.1492x over previous
"""Trainium2 Bass kernel for nn_CausalTemporalAttention.

Reference computation (B=2, S=2048, H=1024, nh=16, hd=64, block=64):
    qkv = x @ w_qkv ; q,k,v heads
    dots = q k^T * hd^-0.5 ; attn = softmax(dots)            (full row)
    attn = attn * block_causal_mask                          (post-softmax)
    out  = (attn @ v) heads-merged @ w_out + b_out
    attention_weights = attn.mean(heads)
Returns (out [B,S,H], attention_weights [B,S,S]).

Sharding: fully data-parallel over (batch, query-blocks). 8 cores; core c
handles batch c//4 and 8 scattered query blocks of 64 rows chosen so the
block-causal masked area is identical across cores:
    j = c%4 : blocks {j, j+4, j+8, j+12, 19-j, 23-j, 27-j, 31-j}
Sorted ascending, local rank r has global block g <= 4r+3, so the
structural key-prefix 256*(r+1) is core-independent; exact masking is
applied via a tiny per-core {0, 1/16} mask input fused into the softmax
normalize op. One SPMD program, no collectives.

Matmuls use float32r (fp32 data, FP22 reduced-precision read) at full PE
rate; attn is cast to bf16 for the attn@V stage.
"""

import numpy as np
import ml_dtypes
from contextlib import ExitStack

import concourse.bass as bass
import concourse.mybir as mybir
import concourse.tile as tile
from concourse.tile_rust import add_dep_helper
from concourse.bass_utils import run_bass_kernel_spmd
from concourse.masks import make_identity

F32 = mybir.dt.float32
F32R = mybir.dt.float32r
BF16 = mybir.dt.bfloat16

B, S, H = 2, 2048, 1024
NH, HD = 16, 64
NB = 64            # block-causal block size (N_BRAIN_AREAS)
SQ = 512           # query rows per core
NBLK = 8           # local query blocks per core (64 rows each)
N_CORES = 8
SCALE = HD ** -0.5  # 0.125

AluOp = mybir.AluOpType
ActFn = mybir.ActivationFunctionType


def blocks_for_quarter(j: int) -> list[int]:
    """8 global query-block indices for core j (ascending, load-balanced)."""
    return sorted([j, j + 4, j + 8, j + 12, 19 - j, 23 - j, 27 - j, 31 - j])


def _split_excess_waits(nc):
    """Walrus caps sync waits per instruction (1 slot on most opcodes; the
    f32r matmul's fused weight-load in particular). Hoist excess waits onto
    standalone same-engine EventSemaphore instructions (2 slots each)."""
    f = nc.m.functions[0]
    for bb in f.blocks:
        out = []
        for ins in bb.instructions:
            si = ins.sync_info
            nw = len(si.on_wait) if si and si.on_wait else 0
            cap = 2 if ins.opcode == "EventSemaphore" else 1
            if nw > cap:
                waits = list(si.on_wait)
                excess, keep = waits[:-cap], waits[-cap:]
                k = 0
                while excess:
                    chunk, excess = excess[:2], excess[2:]
                    out.append(mybir.InstEventSemaphore(
                        name=f"{ins.name}-wsplit{k}", engine=ins.engine,
                        ins=[], outs=[],
                        sync_info=mybir.SyncInfo(on_wait=chunk, on_update=[])))
                    k += 1
                si.on_wait = keep
            out.append(ins)
        bb.instructions = out


def build_kernel(nrep: int = 1, parts: str = "123", aw_eng: str = "mix", do_norm: bool = True, do_av: bool = True, do_exp: bool = True) -> bass.Bass:
    nc = bass.Bass()

    xb = nc.dram_tensor("xb", [S, H], BF16, kind="ExternalInput")
    xq = nc.dram_tensor("xq", [SQ, H], BF16, kind="ExternalInput")
    wqkv = nc.dram_tensor("wqkv", [H, 3 * H], BF16, kind="ExternalInput")
    wout = nc.dram_tensor("wout", [H, H], F32, kind="ExternalInput")  # pre-scaled x16
    bout = nc.dram_tensor("bout", [1, H], F32, kind="ExternalInput")
    ones_in = nc.dram_tensor("ones_in", [1, 128], F32, kind="ExternalInput")
    maskt = nc.dram_tensor("maskt", [S, SQ], BF16, kind="ExternalInput")  # {0,1} transposed mask

    out_q = nc.dram_tensor("out_q", [SQ, H], F32, kind="ExternalOutput")
    aw_q = nc.dram_tensor("aw_q", [SQ, S], F32, kind="ExternalOutput")

    with tile.TileContext(nc) as tc:
     for _rep in range(nrep):
      with ExitStack() as ctx:
        # ---------------- persistent pools ----------------
        const = ctx.enter_context(tc.tile_pool(name="const", bufs=1))
        vpool = ctx.enter_context(tc.tile_pool(name="vpool", bufs=1))
        qtpool = ctx.enter_context(tc.tile_pool(name="qtpool", bufs=1))
        otpool = ctx.enter_context(tc.tile_pool(name="otpool", bufs=1))

        ident_b = const.tile([128, 128], BF16)
        make_identity(nc, ident_b)
        ones_col = const.tile([1, 128], F32R)
        nc.sync.dma_start(out=ones_col, in_=ones_in[:, :].bitcast(F32R))
        bout_sb = const.tile([1, H], F32R)
        nc.sync.dma_start(out=bout_sb, in_=bout[:, :].bitcast(F32R))

        # V in natural layout [token, channel], bf16: v_sb[:, t, :] = V[128t:128t+128, :]
        v_sb = vpool.tile([128, 16, H], BF16)
        # Q^T for local queries: qt_sb[:, c, :] = Q^T[128c:128c+128, :SQ]
        qt_sb = qtpool.tile([128, 8, SQ], BF16)
        # K^T resident: kt_sb[:, c, :] = K^T[128c:128c+128, :]  (head h lives in
        # chunk h//2 at partition offset 64*(h%2) -- matches qt_sb's layout)
        kt_sb = qtpool.tile([128, 8, S], BF16)
        # attn@V output, head-transposed: ot_sb[64*(h%2):.., h//2, :] = (attn_h @ V_h)^T
        ot_sb = otpool.tile([128, 8, SQ], F32R)

        # ---------------- phase 1: projections ----------------
        # weight chunks are loaded with ONE large DMA each via a rearranged AP:
        # wqkv[:, c0:c0+n].rearrange("(c p) n -> p c n") -> [128, 8, n] tile
        def _wchunk(pool, tag, c0, n, bufs=None):
            wc = pool.tile([128, 8, n], BF16, tag=tag, name=tag, bufs=bufs)
            nc.sync.dma_start(
                out=wc, in_=wqkv[:, c0:c0 + n].rearrange("(c p) n -> p c n", p=128))
            return wc

        with ExitStack() as p1:
            psum1 = p1.enter_context(tc.tile_pool(name="psum1", bufs=2, space="PSUM"))
            xload = p1.enter_context(tc.tile_pool(name="xload", bufs=5))
            wstr = p1.enter_context(tc.tile_pool(name="wstr", bufs=2))

            def transpose_512(dst, xt_tiles, c, coff):
                """Transpose 4 [128,128] blocks (column c) of 4 row-tiles into
                dst[:, coff:coff+512] via one batched PSUM tile + one copy."""
                ps = psum1.tile([128, 512], BF16, tag="tp")
                for t4 in range(4):
                    nc.tensor.transpose(ps[:, 128 * t4:128 * (t4 + 1)],
                                        xt_tiles[t4][:, 128 * c:128 * (c + 1)], ident_b)
                nc.vector.tensor_copy(dst[:, coff:coff + 512], ps)

            xtfp = p1.enter_context(tc.tile_pool(name="xtfp", bufs=1))

            # --- x^T for the local queries; Q projection ---
            with ExitStack() as pq:
                xtqp = pq.enter_context(tc.tile_pool(name="xtqp", bufs=1))
                xtq = xtqp.tile([128, 8, SQ], BF16)
                xt_tiles = []
                for t4 in range(4):
                    xt = xload.tile([128, H], BF16, tag="xl")
                    nc.sync.dma_start(out=xt, in_=xq[128 * t4:128 * (t4 + 1), :])
                    xt_tiles.append(xt)
                for c in range(8):
                    transpose_512(xtq[:, c, :], xt_tiles, c, 0)
                for qch in range(8):
                    wc = _wchunk(wstr, "wq", 128 * qch, 128)
                    ps = psum1.tile([128, SQ], F32, tag="mm")
                    for c in range(8):
                        nc.tensor.matmul(ps, wc[:, c, :], xtq[:, c, :],
                                         start=(c == 0), stop=(c == 7))
                    nc.scalar.copy(qt_sb[:, qch, :], ps)

            # --- K and V projections, one 512-token chunk at a time ---
            # (x^T chunk rotates; all of wk/wv stays resident)
            wkc = [_wchunk(wstr, f"wk{kch}", H + 128 * kch, 128, bufs=1) for kch in range(8)]
            wvc = [_wchunk(wstr, f"wv{vh}", 2 * H + 512 * vh, 512, bufs=1) for vh in range(2)]
            for tokc in range(4):
                xt_tiles = []
                for t4 in range(4):
                    xt = xload.tile([128, H], BF16, tag="xl")
                    nc.sync.dma_start(out=xt, in_=xb[512 * tokc + 128 * t4:512 * tokc + 128 * (t4 + 1), :])
                    xt_tiles.append(xt)
                xtc = xtfp.tile([128, 8, 512], BF16, name="xtc")
                for c in range(8):
                    transpose_512(xtc[:, c, :], xt_tiles, c, 0)
                # K^T rows for these 512 tokens -> resident SBUF (bf16)
                for kch in range(8):
                    ps = psum1.tile([128, 512], F32, tag="mm")
                    for c in range(8):
                        nc.tensor.matmul(ps, wkc[kch][:, c, :], xtc[:, c, :],
                                         start=(c == 0), stop=(c == 7))
                    nc.vector.tensor_copy(kt_sb[:, kch, 512 * tokc:512 * (tokc + 1)], ps)
                # V rows (natural layout, bf16)
                for t4 in range(4):
                    for vh in range(2):
                        ps = psum1.tile([128, 512], F32, tag="mm")
                        for c in range(8):
                            nc.tensor.matmul(ps, xtc[:, c, 128 * t4:128 * (t4 + 1)], wvc[vh][:, c, :],
                                             start=(c == 0), stop=(c == 7))
                        nc.scalar.copy(v_sb[:, 4 * tokc + t4, 512 * vh:512 * (vh + 1)], ps)

        # ---------------- phase 2: attention ----------------
        if "2" in parts:
         with ExitStack() as p2:
            dots_ps = p2.enter_context(tc.tile_pool(name="dots_ps", bufs=2, space="PSUM"))
            mm_ps = p2.enter_context(tc.tile_pool(name="mm_ps", bufs=2, space="PSUM"))
            ot_ps = p2.enter_context(tc.tile_pool(name="ot_ps", bufs=2, space="PSUM"))
            expp = p2.enter_context(tc.tile_pool(name="expp", bufs=3))
            attnp = p2.enter_context(tc.tile_pool(name="attnp", bufs=9))
            atp = p2.enter_context(tc.tile_pool(name="atp", bufs=3))
            sums = p2.enter_context(tc.tile_pool(name="sums", bufs=8))
            awpool = p2.enter_context(tc.tile_pool(name="awpool", bufs=1))
            otstg = p2.enter_context(tc.tile_pool(name="otstg", bufs=2))
            maskp = p2.enter_context(tc.tile_pool(name="maskp", bufs=1))

            # transposed mask: maskt_sb[:, kc, :] = maskt[128kc:128(kc+1), :]
            maskt_sb = maskp.tile([128, 16, SQ], BF16)
            for kc in range(16):
                nc.sync.dma_start(out=maskt_sb[:, kc, :], in_=maskt[128 * kc:128 * (kc + 1), :])

            aw_sb = awpool.tile([128, 4, S], F32)
            ot_dmas = []
            tail_ops = []

            def head_front(h):
                """QK -> exp -> normalize -> aw accumulation for head h."""
                hc, hp = h // 2, h % 2
                kht = kt_sb[64 * hp:64 * (hp + 1), hc, :]

                attn_q = []
                for qc in range(4):
                    pq = 512 * (qc + 1)  # structural prefix for this q-chunk
                    qt_h = qt_sb[64 * hp:64 * (hp + 1), hc, 128 * qc:128 * (qc + 1)]
                    halves = []
                    for half in range(2):
                        d = dots_ps.tile([128, 1024], F32, tag="dots")
                        for k2 in range(2):
                            nc.tensor.matmul(
                                d[:, 512 * k2:512 * (k2 + 1)], (qt_h),
                                (kht[:, 1024 * half + 512 * k2:1024 * half + 512 * (k2 + 1)]),
                                start=True, stop=True)
                        ex = expp.tile([128, 1024], BF16, tag="exp")
                        sacc = sums.tile([128, 1], F32, tag="s")
                        if do_exp:
                            nc.scalar.activation(ex, d, ActFn.Exp, scale=SCALE, accum_out=sacc)
                        else:
                            nc.scalar.activation(ex, d, ActFn.Exp, scale=SCALE)
                            nc.vector.memset(sacc, 1.0)
                        halves.append((ex, sacc))
                    stot = sums.tile([128, 1], F32, tag="s")
                    nc.vector.tensor_add(stot, halves[0][1], halves[1][1])
                    recip = sums.tile([128, 1], F32, tag="s")
                    nc.vector.reciprocal(recip, stot)

                    # attn = exp * recip / 16 over the (uniform) structural
                    # prefix; true block-causal masking is fused into the
                    # attn^T evacuation in head_back. attention-weight rows
                    # beyond the true prefix are discarded host-side.
                    attn = attnp.tile([128, S], BF16)
                    for half in range(2):
                        c0, c1 = 1024 * half, min(pq, 1024 * (half + 1))
                        if c1 <= c0 or not do_norm:
                            continue
                        nc.vector.tensor_scalar(
                            out=attn[:, c0:c1], in0=halves[half][0][:, 0:c1 - c0],
                            scalar1=recip, scalar2=1.0 / 16.0,
                            op0=AluOp.mult, op1=AluOp.mult)
                    # attention-weight accumulation (mean over heads)
                    if aw_eng == "none":
                        attn_q.append(attn)
                        continue
                    eng = (nc.gpsimd if qc < 2 else nc.vector) if aw_eng == "mix" else (nc.vector if aw_eng == "dve" else nc.gpsimd)
                    awdst = aw_sb[:, qc, 0:pq]
                    if h == 0:
                        op = eng.tensor_copy(awdst, attn[:, 0:pq])
                    else:
                        op = eng.tensor_add(awdst, attn[:, 0:pq], awdst)
                    if h == NH - 1:
                        tail_ops.append(op)
                    attn_q.append(attn)
                return attn_q

            def head_back(h, attn_q):
                """attn^T (with fused mask) and attn @ V for head h."""
                hc, hp = h // 2, h % 2
                if not do_av:
                    return
                otps = ot_ps.tile([64, SQ], F32, tag="ot")
                for kc in range(16):
                    qc_min = max(0, (kc - 3 + 3) // 4)  # ceil((kc-3)/4)
                    n0 = 128 * qc_min
                    at_ps = mm_ps.tile([128, SQ], BF16, tag="at")
                    for qc in range(qc_min, 4):
                        nc.tensor.transpose(at_ps[:, 128 * qc:128 * (qc + 1)],
                                            attn_q[qc][:, 128 * kc:128 * (kc + 1)], ident_b)
                    at = atp.tile([128, SQ], BF16)
                    nc.vector.tensor_tensor(at[:, n0:SQ], at_ps[:, n0:SQ],
                                            maskt_sb[:, kc, n0:SQ], op=AluOp.mult)
                    nc.tensor.matmul(otps[:, n0:SQ], v_sb[:, kc, 64 * h:64 * (h + 1)],
                                     at[:, n0:SQ], start=(kc == 0), stop=(kc == 15),
                                     skip_group_check=True)
                # evacuate O^T via DMA (engine copies cannot shift partitions)
                otst = otstg.tile([64, SQ], F32R)
                nc.scalar.copy(otst, otps)
                ot_dmas.append(nc.sync.dma_start(out=ot_sb[64 * hp:64 * (hp + 1), hc, :], in_=otst))

            prev = None
            for h in range(NH):
                attn_q = head_front(h)
                if prev is not None:
                    head_back(prev[0], prev[1])
                prev = (h, attn_q)
            head_back(prev[0], prev[1])

            # attention weights out
            if aw_eng != "none":
                for qc in range(4):
                    tail_ops.append(nc.sync.dma_start(out=aw_q[128 * qc:128 * (qc + 1), :], in_=aw_sb[:, qc, :]))

        # ---------------- phase 3: output projection ----------------
        if "3" in parts:
         with ExitStack() as p3:
            psum3 = p3.enter_context(tc.tile_pool(name="psum3", bufs=4, space="PSUM"))
            wstr3 = p3.enter_context(tc.tile_pool(name="wstr3", bufs=2))
            stg3 = p3.enter_context(tc.tile_pool(name="stg3", bufs=3))
            for n2 in range(2):
                pss = []
                for tcn in range(4):
                    ps = psum3.tile([128, 512], F32, tag="op")
                    nc.tensor.matmul(ps, (ones_col), (bout_sb[:, 512 * n2:512 * (n2 + 1)]),
                                     start=True, stop=False, skip_group_check=True)
                    pss.append(ps)
                w = wstr3.tile([128, 8, 512], F32R)
                nc.sync.dma_start(
                    out=w, in_=wout[:, 512 * n2:512 * (n2 + 1)].rearrange("(c p) n -> p c n", p=128).bitcast(F32R))
                for c in range(8):
                    for tcn in range(4):
                        nc.tensor.matmul(pss[tcn], (ot_sb[:, c, 128 * tcn:128 * (tcn + 1)]),
                                         (w[:, c, :]), start=False, stop=(c == 7), skip_group_check=True)
                for tcn in range(4):
                    st = stg3.tile([128, 512], F32)
                    nc.scalar.copy(st, pss[tcn])
                    nc.sync.dma_start(out=out_q[128 * tcn:128 * (tcn + 1), 512 * n2:512 * (n2 + 1)], in_=st)

    _split_excess_waits(nc)
    return nc


_COMPILED = None


def _get_nc():
    global _COMPILED
    if _COMPILED is None:
        _COMPILED = build_kernel()
    return _COMPILED


def _host_maskt(j: int) -> np.ndarray:
    """[S, SQ] bf16 transposed mask: 1 where key-block <= global query-block."""
    g = np.repeat(np.array(blocks_for_quarter(j)), NB)        # [SQ] global block per local col
    kb = np.arange(S) // NB                                   # [S] key block per row
    return (kb[:, None] <= g[None, :]).astype(ml_dtypes.bfloat16)


def run_shards(x, w_qkv, w_out, b_out, **spmd_kwargs):
    """Shard inputs, run the SPMD kernel, reassemble. Returns (output, attnw, bkr)."""
    x = np.ascontiguousarray(np.asarray(x, dtype=np.float32).astype(ml_dtypes.bfloat16))
    w_qkv = np.ascontiguousarray(np.asarray(w_qkv, dtype=np.float32).astype(ml_dtypes.bfloat16))
    w_out16 = np.ascontiguousarray(np.asarray(w_out, dtype=np.float32) * 16.0)
    b_out = np.ascontiguousarray(np.asarray(b_out, dtype=np.float32)).reshape(1, H)

    nc = _get_nc()
    in_maps = []
    for c in range(N_CORES):
        b, j = c // 4, c % 4
        rows = np.concatenate([np.arange(NB * g, NB * (g + 1)) for g in blocks_for_quarter(j)])
        in_maps.append({
            "xb": x[b],
            "xq": np.ascontiguousarray(x[b][rows]),
            "wqkv": w_qkv,
            "wout": w_out16,
            "bout": b_out,
            "ones_in": np.ones((1, 128), np.float32),
            "maskt": _host_maskt(j),
        })
    res = run_bass_kernel_spmd(nc, in_maps, core_ids=list(range(N_CORES)), **spmd_kwargs)

    output = np.empty((B, S, H), dtype=np.float32)
    attnw = np.zeros((B, S, S), dtype=np.float32)
    for c in range(N_CORES):
        b, j = c // 4, c % 4
        o = np.asarray(res.results[c]["out_q"])
        a = np.asarray(res.results[c]["aw_q"])
        for r, g in enumerate(blocks_for_quarter(j)):
            rows = slice(NB * g, NB * (g + 1))
            output[b, rows, :] = o[NB * r:NB * (r + 1), :]
            pref = NB * (g + 1)
            attnw[b, rows, :pref] = a[NB * r:NB * (r + 1), :pref]
    return output, attnw, res


def kernel(x, w_qkv, w_out, b_out):
    output, attnw, _ = run_shards(x, w_qkv, w_out, b_out)
    return output, attnw
